# revision 21
# baseline (speedup 1.0000x reference)
"""BiDAF attention-flow kernel for Trainium2 (8 NeuronCores, data-parallel).

Self-contained: hardcodes shapes B,C,Q,H2 = 64,512,64,256; n_labels=2.
kernel(**inputs) takes full unsharded inputs, shards batch over 8 cores,
runs one SPMD Bass/Tile kernel, gathers [8,2] per core -> [64,2].

Per-core math (8 examples, bf16 compute, fp32 accumulation):
  S = c @ diag(w_m) @ q^T + (c@w_c)[:,None] + (q@w_q)[None,:]
    - the c@w_c term folds into the matmul rhs (rhs = w_m*q^T + w_c),
    - the q@w_q term rides in via a K=1 all-ones broadcast matmul.
  P = exp(S) unstabilized (|S| is O(1) for this distribution), so
  row-softmax needs only row-sums, and b_att = softmax(max_j S) is just
  Pmax/sum(Pmax) with Pmax = max_j P  (exp is monotone).

Layouts (vs the v0 baseline):
  - c loads 4-row-packed: partition p = i//4, free chunk r = i%4, so each
    SWDGE packet reads 4 contiguous HBM rows (4KB) instead of 1KB.  All
    "context chunks" downstream are residue classes i%4 (any consistent
    permutation of i is fine: pools/q2c/softmax are row-perm invariant).
  - q loads once, natural row order: partition = 64*(e%2)+j, free = e//2.
    Pairs (2g, 2g+1) then sit on partition halves exactly as the c2q
    tile-split wants, and one PE transpose yields q^T for a whole pair.
  - All q-side prep (q^T, w_q.q, rhs_qm) runs up front while c streams in.
  - q2c = b_att @ c is computed with c as the matmul stationary so the
    result lands directly as a [d, e] column (no extra transposes).
  - b_att normalizers via gpsimd.partition_all_reduce (PE stays free).
  - max_i c / min_i c pools run right after each pair's c^T lands,
    overlapping the tail of the DMA load phase.
"""

import os
import sys

for _p in ("/opt/trn_rl_repo", "/opt/pypackages"):
    if os.path.isdir(_p) and _p not in sys.path:
        sys.path.insert(0, _p)

import numpy as np

import concourse.bass as bass
import concourse.bacc as bacc
import concourse.tile as tile
import concourse.mybir as mybir
from concourse import bass_isa
from concourse.bass_utils import run_bass_kernel_spmd
from concourse.masks import make_identity
from concourse.tile_rust import add_dep_helper

F32 = mybir.dt.float32
BF16 = mybir.dt.bfloat16
AX = mybir.AxisListType
OP = mybir.AluOpType
AF = mybir.ActivationFunctionType

DEBUG_DUMP = False

N_CORES = 8
B, C, Q, H2 = 64, 512, 64, 256
NL = 2
EX = B // N_CORES          # examples per core = 8
R = 4                      # context residue chunks (i % 4)
DH = H2 // 128             # feature chunks of 128 = 2
NK = 4 * DH                # final feature chunks (4 pieces x DH) = 8
NP = EX // 2               # pairs = 4


def _body(tc, ctx, fd, fq, wsim, wlab, blab, out):
    nc = tc.nc

    consts = ctx.enter_context(tc.tile_pool(name="consts", bufs=1))
    bigbuf = ctx.enter_context(tc.tile_pool(name="bigbuf", bufs=1))
    scr_pool = ctx.enter_context(tc.tile_pool(name="scr", bufs=8))
    s3_pool = ctx.enter_context(tc.tile_pool(name="s3", bufs=3))
    den_pool = ctx.enter_context(tc.tile_pool(name="den", bufs=3))
    sb_small = ctx.enter_context(tc.tile_pool(name="small", bufs=1))

    ps_s_pool = ctx.enter_context(tc.tile_pool(name="pss", bufs=2, space="PSUM"))
    ps_c2q_pool = ctx.enter_context(tc.tile_pool(name="psc", bufs=1, space="PSUM"))
    ps_misc_pool = ctx.enter_context(tc.tile_pool(name="psm", bufs=2, space="PSUM"))

    # ---- small consts: fp32, no cast -> sync HWDGE queue (frees gpsimd) ----
    w_sb = consts.tile([128, 6], F32)          # col = t*2+dh; t: 0=w_c 1=w_q 2=w_m
    nc.sync.dma_start(w_sb[:, :], wsim[:].rearrange("(t dh p) -> p (t dh)", dh=DH, p=128))
    wlab_sb = consts.tile([128, NK, NL], F32)  # chunk k = piece*DH+dh
    nc.sync.dma_start(wlab_sb[:, :, :], wlab[:, :].rearrange("(k p) l -> p k l", p=128))
    b_sb = consts.tile([1, NL], F32)
    nc.sync.dma_start(b_sb[0:1, :], blab[:].rearrange("(o l) -> o l", o=1))
    # q natural: partition = 64*(e%2)+j, free g = e//2.  Cast-loaded on the
    # SWDGE FIFO AHEAD of c so the query-side prep starts immediately; the
    # consts above ride the (otherwise empty) sync HWDGE queue.
    q_bf = bigbuf.tile([128, NP, H2], BF16)
    nc.gpsimd.dma_start(q_bf[:, :, :], fq[:, :, :].rearrange("(g h) j d -> (h j) g d", h=2))

    wq_bf = consts.tile([128, DH], BF16)       # w_q as bf16 matmul operand
    nc.vector.tensor_copy(wq_bf[:, :], w_sb[:, 2:4])
    ones_bf = consts.tile([1, 128], BF16)      # K=1 broadcast lhsT
    nc.vector.memset(ones_bf[0:1, :], 1.0)
    ones_f32 = consts.tile([1, 128], F32)      # bias broadcast lhsT
    nc.vector.memset(ones_f32[0:1, :], 1.0)
    id_bf = consts.tile([128, 128], BF16)      # identity for PE transposes
    make_identity(nc, id_bf[:, :])

    # ---- big input: c cast-load fp32 -> bf16 (SWDGE), 4-row-packed ----
    # partition p = i//4, chunk r = i%4: each packet reads 4 contiguous
    # 1KB HBM rows.  Chained per-pair so pair g's data (and compute)
    # finishes early instead of everything completing together.
    c_nat = bigbuf.tile([128, EX, R, H2], BF16)
    for g in range(NP):
        # no gating: the SWDGE queue is a FIFO, so group g's packets
        # complete before group g+1's regardless
        nc.gpsimd.dma_start(
            c_nat[:, 2 * g:2 * g + 2, :, :],
            fd[2 * g:2 * g + 2, :, :].rearrange("e (p r) d -> p e r d", r=R),
        )

    def pe_transpose_group(psum_view, srcs):
        """Transpose each [128|64,128] src into psum_view[:, k, :] via PE."""
        first = None
        for k, src in enumerate(srcs):
            mm = nc.tensor.matmul(
                psum_view[:, k, :], src, id_bf[0:src.shape[0], 0:src.shape[0]],
                is_transpose=True,
                start=(first is None), stop=(k == len(srcs) - 1),
                skip_group_check=True,
            )
            if first is None:
                first = mm
            else:
                add_dep_helper(mm.ins, first.ins, sync=False, reason="bank order")
        return first

    # ---- persistent tiles ----
    c_T = bigbuf.tile([128, EX, DH, C], BF16)       # col = r*128 + p
    q_T = bigbuf.tile([128, NP, DH, 128], BF16)     # col = 64*slot + j
    rhs_qm = bigbuf.tile([128, NP, DH, 128], BF16)  # w_m*q^T + w_c
    qwrow = sb_small.tile([1, NP * 128], BF16)      # q @ w_q, col = g*128+64*slot+j

    pm_col = sb_small.tile([128, EX * R], BF16)     # Pmax^T, col = e*R+r
    final_f = sb_small.tile([128, NK * EX], F32)    # col = (piece*DH+dh)*EX + e
    cmin_f = sb_small.tile([128, DH * EX], F32)     # col = dh*EX + e
    r_sb = sb_small.tile([128, EX], F32)            # 1/sum(pm), all partitions
    bsum = sb_small.tile([128, EX * R], F32)        # partition_all_reduce out
    q2cT_sb = sb_small.tile([128, EX, DH], F32)
    out_sb = sb_small.tile([EX, NL], F32)

    P_all = sb_small.tile([128, R, EX, Q], BF16)
    Pn_all = sb_small.tile([128, R, EX, Q], BF16)
    PT_all = sb_small.tile([128, NP, R, 128], BF16)  # row = 64*slot+j, col = r*128+p

    fview = final_f[:, :].rearrange("p (pc dh e) -> p pc dh e", pc=4, dh=DH)
    cminv = cmin_f[:, :].rearrange("p (dh e) -> p dh e", dh=DH)

    # ---------- phase Q: all query-side prep, before c arrives ----------
    for g in range(NP):
        tp = ps_misc_pool.tile([128, DH, 128], BF16, tag="misc")
        pe_transpose_group(
            tp, [q_bf[:, g, dh * 128:(dh + 1) * 128] for dh in range(DH)])
        nc.scalar.copy(q_T[:, g, :, :], tp[:, :, :])
    for dh in range(DH):
        # rhs_qm = w_m * q^T + w_c, batched across all pairs
        nc.scalar.activation(
            rhs_qm[:, :, dh, :], q_T[:, :, dh, :], AF.Identity,
            bias=w_sb[:, 0 + dh:1 + dh], scale=w_sb[:, 4 + dh:5 + dh],
        )
    ps_qw = ps_misc_pool.tile([1, 512], F32, tag="misc")
    for dh in range(DH):
        nc.tensor.matmul(
            ps_qw[0:1, :], wq_bf[:, dh:dh + 1], q_T[:, :, dh, :],
            start=(dh == 0), stop=(dh == DH - 1),
        )
    nc.vector.tensor_copy(qwrow[0:1, :], ps_qw[0:1, :])

    # ---------- per-pair stages ----------
    def stage_A(g):
        """c^T transposes for pair g (PE -> PSUM -> SBUF via ACT)."""
        for e in (2 * g, 2 * g + 1):
            for dh in range(DH):
                tp = ps_misc_pool.tile([128, R, 128], BF16, tag="misc")
                pe_transpose_group(
                    tp,
                    [c_nat[:, e, r, dh * 128:(dh + 1) * 128] for r in range(R)],
                )
                nc.scalar.copy(c_T[:, e, dh, :], tp[:, :, :])

    def stage_A2(g):
        """max_i c and min_i c pools (independent of S; overlaps loads).
        Fold trees run on the otherwise-idle gpsimd; only the final 1x
        reduce (free-dim, gpsimd can't) stays on the vector engine."""
        e0 = 2 * g
        eP = slice(e0, e0 + 2)
        cT_p = c_T[:, eP, :, :]                 # [128, 2, DH, C]
        for op, dst in ((OP.max, fview[:, 0, :, eP].rearrange("p dh e -> p e dh")),
                        (OP.min, cminv[:, :, eP].rearrange("p dh e -> p e dh"))):
            f1 = scr_pool.tile([128, 2, DH, 256], BF16, tag="bigA")
            nc.vector.tensor_tensor(
                f1[:, :, :, :], cT_p[:, :, :, 0:256], cT_p[:, :, :, 256:512], op=op)
            f2 = scr_pool.tile([128, 2, DH, 128], BF16, tag="bigB")
            nc.vector.tensor_tensor(
                f2[:, :, :, :], f1[:, :, :, 0:128], f1[:, :, :, 128:256], op=op)
            f3 = scr_pool.tile([128, 2, DH, 64], BF16, tag="bigC")
            nc.vector.tensor_tensor(
                f3[:, :, :, :], f2[:, :, :, 0:64], f2[:, :, :, 64:128], op=op)
            nc.vector.tensor_reduce(dst, f3[:, :, :, :], axis=AX.X, op=op)

    def stage_B(g):
        """S matmuls + softmax pieces for pair g."""
        e0 = 2 * g
        ps_s = ps_s_pool.tile([128, R, 2, Q], F32)
        first_mm = None
        for slot in range(2):
            e = e0 + slot
            for r in range(R):
                for dh in range(DH):
                    mm = nc.tensor.matmul(
                        ps_s[:, r, slot, :],
                        c_T[:, e, dh, r * 128:(r + 1) * 128],
                        rhs_qm[:, g, dh, slot * 64:(slot + 1) * 64],
                        start=(first_mm is None), stop=False,
                        skip_group_check=True,
                    )
                    if first_mm is None:
                        first_mm = mm
                    else:
                        add_dep_helper(mm.ins, first_mm.ins, sync=False,
                                       reason="bank clear order")
        for r in range(R):
            mm = nc.tensor.matmul(
                ps_s[:, r, :, :].rearrange("p s j -> p (s j)"),
                ones_bf[0:1, :], qwrow[0:1, g * 128:(g + 1) * 128],
                start=False, stop=(r == R - 1),
                skip_group_check=True,
            )
            add_dep_helper(mm.ins, first_mm.ins, sync=False,
                           reason="bank clear order")

        pview = P_all[:, :, e0:e0 + 2, :]
        nc.scalar.activation(pview, ps_s[:, :, :, :], AF.Exp)
        # den / pm are 1x-capped reduces: run them on the idle gpsimd
        den = den_pool.tile([128, R, 2], F32)
        nc.vector.reduce_sum(den[:, :, :], pview, axis=AX.X)
        nc.vector.tensor_reduce(
            pm_col[:, e0 * R:(e0 + 2) * R].rearrange("p (e r) -> p r e", r=R),
            pview, axis=AX.X, op=OP.max,
        )
        rden = den_pool.tile([128, R, 2], F32, tag="rden")
        nc.vector.reciprocal(rden[:, :, :], den[:, :, :])
        nc.vector.tensor_tensor(
            Pn_all[:, :, e0:e0 + 2, :], pview,
            rden[:, :, :].unsqueeze(3).broadcast_to([128, R, 2, Q]),
            op=OP.mult,
        )

    def stage_Q2C(g):
        """q2c matmuls with raw (unnormalized) pm: only needs B(g)'s pm,
        so the PE can run these while softmax/PT of pair g continues."""
        e0 = 2 * g
        ps_q2 = ps_misc_pool.tile([128, 2, DH], F32, tag="misc")
        for slot in range(2):
            e = e0 + slot
            for dh in range(DH):
                for r in range(R):
                    nc.tensor.matmul(
                        ps_q2[:, slot, dh:dh + 1],
                        c_nat[:, e, r, dh * 128:(dh + 1) * 128],
                        pm_col[:, e * R + r:e * R + r + 1],
                        start=(r == 0), stop=(r == R - 1),
                        skip_group_check=True,
                    )
        nc.scalar.copy(q2cT_sb[:, e0:e0 + 2, :], ps_q2[:, :, :])

    def stage_C(g):
        """P^T transpose for pair g."""
        e0 = 2 * g
        tp = ps_misc_pool.tile([128, R, 128], BF16, tag="misc")
        pe_transpose_group(tp, [Pn_all[:, r, e0:e0 + 2, :] for r in range(R)])
        nc.scalar.copy(PT_all[:, g, :, :], tp[:, :, :])

    def stage_D(g):
        """c2q + dependent max-pools + q2c for pair g."""
        e0 = 2 * g
        eP = slice(e0, e0 + 2)
        ps_c2q = ps_c2q_pool.tile([128, 2, DH, C], F32)   # 4 banks
        for slot in range(2):
            for dh in range(DH):
                nc.tensor.matmul(
                    ps_c2q[:, slot, dh, :],
                    q_bf[slot * 64:slot * 64 + 64, g, dh * 128:(dh + 1) * 128],
                    PT_all[slot * 64:slot * 64 + 64, g, :, :],
                    start=True, stop=True,
                    tile_position=(slot * 64, 0),
                )

        # c2q PSUM -> SBUF (ACT), then folds/products run at DVE 2x
        c2q_sb = scr_pool.tile([128, 2, DH, C], BF16, tag="c2q")
        nc.scalar.copy(c2q_sb[:, :, :, :], ps_c2q[:, :, :, :])
        prod = scr_pool.tile([128, 2, DH, C], BF16, tag="prod")
        nc.vector.tensor_tensor(
            prod[:, :, :, :], c_T[:, eP, :, :], c2q_sb[:, :, :, :], op=OP.mult)
        scrA = scr_pool.tile([128, 2, DH, 256], BF16, tag="scrA")
        nc.vector.tensor_tensor(
            scrA[:, :, :, :], c2q_sb[:, :, :, 0:256], c2q_sb[:, :, :, 256:512],
            op=OP.max)
        scrB = scr_pool.tile([128, 2, DH, 128], BF16, tag="scrB")
        nc.vector.tensor_tensor(
            scrB[:, :, :, :], scrA[:, :, :, 0:128], scrA[:, :, :, 128:256], op=OP.max)
        nc.vector.tensor_reduce(
            fview[:, 1, :, eP].rearrange("p dh e -> p e dh"),
            scrB[:, :, :, :], axis=AX.X, op=OP.max)

        # piece 2 folds
        scrA2 = scr_pool.tile([128, 2, DH, 256], BF16, tag="scrA2")
        scrB2 = scr_pool.tile([128, 2, DH, 128], BF16, tag="scrB2")
        nc.vector.tensor_tensor(
            scrA2[:, :, :, :], prod[:, :, :, 0:256], prod[:, :, :, 256:512], op=OP.max)
        nc.vector.tensor_tensor(
            scrB2[:, :, :, :], scrA2[:, :, :, 0:128], scrA2[:, :, :, 128:256], op=OP.max)
        nc.vector.tensor_reduce(
            fview[:, 2, :, eP].rearrange("p dh e -> p e dh"),
            scrB2[:, :, :, :], axis=AX.X, op=OP.max)

    def stage_P3():
        """b_att normalizers + piece 3 for ALL pairs, batched at the end.
        piece 3: max_i (c * q2c) = max(q2c*cmax, q2c*cmin); q2c was
        accumulated with raw pm, so fold in the 1/sum(pm) scale here."""
        nc.gpsimd.partition_all_reduce(
            bsum[:, :], pm_col[:, :],
            channels=128, reduce_op=bass_isa.ReduceOp.add,
        )
        sb2 = den_pool.tile([128, EX], F32, tag="sb2")
        nc.vector.reduce_sum(
            sb2[:, :], bsum[:, :].rearrange("p (e r) -> p e r", r=R), axis=AX.X)
        nc.vector.reciprocal(r_sb[:, :], sb2[:, :])
        s3a = s3_pool.tile([128, EX, DH], F32, tag="s3a")
        s3b = s3_pool.tile([128, EX, DH], F32, tag="s3b")
        q2n = s3_pool.tile([128, EX, DH], F32, tag="q2n")
        nc.vector.tensor_tensor(
            q2n[:, :, :], q2cT_sb[:, :, :],
            r_sb[:, :].unsqueeze(2).broadcast_to([128, EX, DH]),
            op=OP.mult)
        nc.vector.tensor_tensor(
            s3a[:, :, :], q2n[:, :, :],
            fview[:, 0, :, :].rearrange("p dh e -> p e dh"), op=OP.mult)
        nc.vector.tensor_tensor(
            s3b[:, :, :], q2n[:, :, :],
            cminv[:, :, :].rearrange("p dh e -> p e dh"), op=OP.mult)
        nc.vector.tensor_tensor(
            fview[:, 3, :, :].rearrange("p dh e -> p e dh"),
            s3a[:, :, :], s3b[:, :, :], op=OP.max)

    # ---- software-pipelined emission: pair g+1 transposes fill the PE
    # while pair g's softmax runs on ACT/DVE ----
    stage_A(0)
    stage_A2(0)
    stage_A(1)
    stage_B(0)
    stage_Q2C(0)
    stage_A2(1)
    stage_C(0)
    stage_A(2)
    stage_B(1)
    stage_Q2C(1)
    stage_A2(2)
    stage_D(0)
    stage_C(1)
    stage_A(3)
    stage_B(2)
    stage_Q2C(2)
    stage_A2(3)
    stage_D(1)
    stage_C(2)
    stage_B(3)
    stage_Q2C(3)
    stage_D(2)
    stage_C(3)
    stage_P3()
    stage_D(3)

    # ---------- final: out = max-pooled features @ w_label + b ----------
    ps_out = ps_misc_pool.tile([128, 512], F32, tag="misc")
    for k in range(NK):
        nc.tensor.matmul(
            ps_out[0:EX, 0:NL], final_f[:, k * EX:(k + 1) * EX], wlab_sb[:, k, :],
            start=(k == 0), stop=False, skip_group_check=True,
        )
    nc.tensor.matmul(
        ps_out[0:EX, 0:NL], ones_f32[0:1, 0:EX], b_sb[0:1, :],
        start=False, stop=True, skip_group_check=True,
    )
    nc.vector.tensor_copy(out_sb[:, :], ps_out[0:EX, 0:NL])
    nc.sync.dma_start(out[:, :], out_sb[:, :])

    if DEBUG_DUMP:
        for name, tl, shp, dt in [
            ("dbg_final", final_f, [128, NK * EX], F32),
            ("dbg_cmin", cmin_f, [128, DH * EX], F32),
            ("dbg_pm", pm_col, [128, EX * R], BF16),
            ("dbg_r", r_sb, [128, EX], F32),
            ("dbg_q2cT", q2cT_sb, [128, EX * DH], F32),
        ]:
            t = nc.dram_tensor(name, shp, dt, kind="ExternalOutput")
            nc.sync.dma_start(t[:, :], tl[:, :].rearrange("p a b -> p (a b)")
                              if len(tl.shape) == 3 else tl[:, :])


def build_nc():
    nc = bacc.Bacc("TRN2", target_bir_lowering=False, debug=False)
    fd = nc.dram_tensor("fd", [EX, C, H2], F32, kind="ExternalInput")
    fq = nc.dram_tensor("fq", [EX, Q, H2], F32, kind="ExternalInput")
    wsim = nc.dram_tensor("wsim", [3 * H2], F32, kind="ExternalInput")
    wlab = nc.dram_tensor("wlab", [4 * H2, NL], F32, kind="ExternalInput")
    blab = nc.dram_tensor("blab", [NL], F32, kind="ExternalInput")
    out = nc.dram_tensor("out", [EX, NL], F32, kind="ExternalOutput")

    from contextlib import ExitStack
    with tile.TileContext(nc) as tc:
        with ExitStack() as ctx:
            _body(tc, ctx, fd[:, :, :], fq[:, :, :], wsim[:], wlab[:, :], blab[:], out[:, :])
    nc.compile()
    return nc


_NC_CACHE = None


def run(inputs, trace=False):
    global _NC_CACHE
    if _NC_CACHE is None:
        _NC_CACHE = build_nc()
    nc = _NC_CACHE

    fd = np.ascontiguousarray(np.asarray(inputs["feature_document"], dtype=np.float32))
    fq = np.ascontiguousarray(np.asarray(inputs["feature_query"], dtype=np.float32))
    wsim = np.ascontiguousarray(np.asarray(inputs["w_sim"], dtype=np.float32))
    wlab = np.ascontiguousarray(np.asarray(inputs["w_label"], dtype=np.float32))
    blab = np.ascontiguousarray(np.asarray(inputs["b_label"], dtype=np.float32))

    in_maps = []
    for core in range(N_CORES):
        sl = slice(core * EX, (core + 1) * EX)
        in_maps.append({
            "fd": fd[sl], "fq": fq[sl],
            "wsim": wsim, "wlab": wlab, "blab": blab,
        })
    res = run_bass_kernel_spmd(nc, in_maps, list(range(N_CORES)), trace=trace)
    outs = np.concatenate([np.asarray(res.results[i]["out"]) for i in range(N_CORES)], axis=0)
    return outs.astype(np.float32), res


def kernel(**inputs):
    outs, _ = run(inputs, trace=False)
    return outs
